# revision 6
# baseline (speedup 1.0000x reference)
"""Exphormer edge-attention kernel for 8 Trainium2 NeuronCores.

Strategy (v2 — batched dma_gather):
  - Host: bucket edges by 128-node destination window (49 windows/core).
    Within each window, edges are split by src half (< 32768 vs >=) so
    that gather indices fit int16; each half is padded to a uniform
    block count (bl / bh) across all windows and cores.
  - Device (per core):
      prologue: project K|V for ALL nodes (kv_lo/kv_hi bf16 tables in
        DRAM) and Q for the core's OWN 6272 dst nodes (q_loc table).
      main loop, per window:
        * three dma_gathers (kv lo half by src, kv hi half by src,
          Q by core-local dst) — one SWDGE instruction each, thousands
          of 512B/256B descriptors, rotating across 4 queues
        * one HWDGE load of the window's edge_attr (bf16, pre-cast on
          host)
        * per 4-block quad: Eh = ea @ WE on PE; t = (K*Q)*Eh on DVE;
          per-head reduce -> clip -> exp(0.25 x) on ACT; msg = V*score;
          scatter via onehot(drel)^T @ [msg|score] accumulated in PSUM
        * epilogue: out = wV / (Z + 1e-6), DMA to output slice.
  - Host: concatenate core outputs, trim padding rows.
"""

import math
import os
import sys
from contextlib import ExitStack

import numpy as np

for _p in ("/opt/trn_rl_repo", "/root/.axon_site/_ro/trn_rl_repo"):
    if os.path.isdir(_p) and _p not in sys.path:
        sys.path.insert(0, _p)

import ml_dtypes  # noqa: E402

import concourse.bass as bass  # noqa: E402
import concourse.tile as tile  # noqa: E402
from concourse import bacc, mybir  # noqa: E402
from concourse.bass_utils import run_bass_kernel_spmd  # noqa: E402

F32 = mybir.dt.float32
BF16 = mybir.dt.bfloat16
I16 = mybir.dt.int16
ALU = mybir.AluOpType
ACTF = mybir.ActivationFunctionType

N_NODES = 50000
N_EDGES = 1600000
DIM = 128
H = 8
D = 16
N_CORES = 8
P = 128
QUAD = 4
HALF = 32768  # node-id split so gather indices fit int16

LAST_EXEC_NS = None
LAST_NC = None
LAST_IN_MAPS = None


def build_program(n_cores, wpc, bl, bh, ablate=frozenset(), repeat=1):
    bpg = bl + bh
    nb = wpc * bpg                      # blocks per core
    n_pad = 392 * P                     # padded global node count
    n_loc = wpc * P                     # per-core dst nodes
    hi_rows = n_pad - HALF

    nc = bacc.Bacc(
        "TRN2", target_bir_lowering=False, debug=False, num_devices=n_cores,
        num_swdge_queues=4,
    )

    ea_t = nc.dram_tensor("eaT", [P, nb * P], BF16, kind="ExternalInput").ap()
    slo_t = nc.dram_tensor("sloT", [P, wpc * bl * 8], I16, kind="ExternalInput").ap()
    shi_t = nc.dram_tensor("shiT", [P, wpc * bh * 8], I16, kind="ExternalInput").ap()
    qix_t = nc.dram_tensor("qixT", [P, wpc * bpg * 8], I16, kind="ExternalInput").ap()
    drel_t = nc.dram_tensor("drelT", [P, nb], BF16, kind="ExternalInput").ap()
    h_t = nc.dram_tensor("hTb", [P, n_pad], BF16, kind="ExternalInput").ap()
    hq_t = nc.dram_tensor("hqTb", [P, n_loc], BF16, kind="ExternalInput").ap()
    w4 = nc.dram_tensor("w4", [P, 4 * DIM], F32, kind="ExternalInput").ap()
    iota = nc.dram_tensor("iota4", [P, QUAD * P], BF16, kind="ExternalInput").ap()
    out = nc.dram_tensor("out", [wpc * P, DIM], F32, kind="ExternalOutput").ap()

    kv_lo = nc.dram_tensor("kv_lo", [HALF, 2 * DIM], BF16).ap()
    kv_hi = nc.dram_tensor("kv_hi", [hi_rows, 2 * DIM], BF16).ap()
    q_loc = nc.dram_tensor("q_loc", [n_loc, DIM], BF16).ap()

    with tile.TileContext(nc) as tc, ExitStack() as ctx:
        singles = ctx.enter_context(tc.tile_pool(name="singles", bufs=1))
        tbl = ctx.enter_context(tc.tile_pool(name="tbl", bufs=3))
        tbl_ps = ctx.enter_context(tc.tile_pool(name="tbl_ps", bufs=2, space="PSUM"))
        win_pool = ctx.enter_context(tc.tile_pool(name="win", bufs=2))
        stream = ctx.enter_context(tc.tile_pool(name="stream", bufs=5))
        eh_ps = ctx.enter_context(tc.tile_pool(name="eh_ps", bufs=3, space="PSUM"))
        acc_ps = ctx.enter_context(tc.tile_pool(name="acc_ps", bufs=2, space="PSUM"))
        evac = ctx.enter_context(tc.tile_pool(name="evac", bufs=3))

        # ---- resident constants / index tables ----
        w4f = singles.tile([P, 4 * DIM], F32)
        nc.sync.dma_start(out=w4f[:], in_=w4[:])
        w4b = singles.tile([P, 4 * DIM], BF16)
        nc.scalar.activation(out=w4b[:], in_=w4f[:], func=ACTF.Copy)
        iot = singles.tile([P, QUAD, P], BF16)
        nc.sync.dma_start(out=iot[:], in_=iota[:].rearrange("p (q n) -> p q n", q=QUAD))
        slo_all = singles.tile([P, wpc * bl * 8], I16)
        nc.sync.dma_start(out=slo_all[:], in_=slo_t[:])
        shi_all = singles.tile([P, wpc * bh * 8], I16)
        nc.sync.dma_start(out=shi_all[:], in_=shi_t[:])
        qix_all = singles.tile([P, wpc * bpg * 8], I16)
        nc.sync.dma_start(out=qix_all[:], in_=qix_t[:])
        drel_all = singles.tile([P, nb], BF16)
        nc.sync.dma_start(out=drel_all[:], in_=drel_t[:])

        swdge_ctr = [0]

        def next_q():
            q = swdge_ctr[0] % 4
            swdge_ctr[0] += 1
            return q

        def emit_rep():
            # ---- prologue: K|V tables for all nodes, Q for local nodes ----
            if "notables" not in ablate:
                for t in range(n_pad // P):
                    hb = tbl.tile([P, P], BF16, tag="hb")
                    nc.sync.dma_start(out=hb[:], in_=h_t[:, t * P : (t + 1) * P])
                    kvp = tbl_ps.tile([P, 2 * DIM], F32, space="PSUM", tag="kvp")
                    nc.tensor.matmul(
                        out=kvp[:], lhsT=hb[:], rhs=w4b[:, DIM : 3 * DIM],
                        start=True, stop=True,
                    )
                    kvt = tbl.tile([P, 2 * DIM], BF16, tag="kvt")
                    nc.vector.tensor_copy(out=kvt[:], in_=kvp[:])
                    if t < HALF // P:
                        nc.sync.dma_start(
                            out=kv_lo[t * P : (t + 1) * P, :], in_=kvt[:]
                        )
                    else:
                        t2 = t - HALF // P
                        nc.sync.dma_start(
                            out=kv_hi[t2 * P : (t2 + 1) * P, :], in_=kvt[:]
                        )
                for t in range(wpc):
                    hb = tbl.tile([P, P], BF16, tag="hb")
                    nc.sync.dma_start(out=hb[:], in_=hq_t[:, t * P : (t + 1) * P])
                    qp = tbl_ps.tile([P, 2 * DIM], F32, space="PSUM", tag="kvp")
                    nc.tensor.matmul(
                        out=qp[:, 0:DIM], lhsT=hb[:], rhs=w4b[:, 0:DIM],
                        start=True, stop=True,
                    )
                    qt = tbl.tile([P, DIM], BF16, tag="qt")
                    nc.scalar.activation(out=qt[:], in_=qp[:, 0:DIM], func=ACTF.Copy)
                    nc.sync.dma_start(out=q_loc[t * P : (t + 1) * P, :], in_=qt[:])

            # ---- main loop ----
            for w in range(wpc):
                kvlo = win_pool.tile([P, bl, 2 * DIM], BF16, tag="kvlo")
                if "nogather" not in ablate:
                    nc.gpsimd.dma_gather(
                        kvlo[:], kv_lo[:],
                        slo_all[:, w * bl * 8 : (w + 1) * bl * 8],
                        bl * P, bl * P, 2 * DIM,
                        single_packet=False, queue_num=next_q(),
                    )
                kvhi = win_pool.tile([P, bh, 2 * DIM], BF16, tag="kvhi")
                if "nogather" not in ablate:
                    nc.gpsimd.dma_gather(
                        kvhi[:], kv_hi[:],
                        shi_all[:, w * bh * 8 : (w + 1) * bh * 8],
                        bh * P, bh * P, 2 * DIM,
                        single_packet=False, queue_num=next_q(),
                    )
                qg = win_pool.tile([P, bpg, DIM], BF16, tag="qg")
                if "noqgather" not in ablate:
                    nc.gpsimd.dma_gather(
                        qg[:], q_loc[:],
                        qix_all[:, w * bpg * 8 : (w + 1) * bpg * 8],
                        bpg * P, bpg * P, DIM,
                        single_packet=False, queue_num=next_q(),
                    )
                eat = win_pool.tile([P, bpg * P], BF16, tag="eat")
                nc.sync.dma_start(
                    out=eat[:], in_=ea_t[:, w * bpg * P : (w + 1) * bpg * P]
                )

                acc = acc_ps.tile([P, DIM + H], F32, space="PSUM", tag="acc")
                nsteps = 0
                total_steps = bl + bh
                for half, hbl, kvt_ in ((0, bl, kvlo), (1, bh, kvhi)):
                    boff = 0 if half == 0 else bl
                    nquads = (hbl + QUAD - 1) // QUAD
                    for qd in range(nquads):
                        k0 = qd * QUAD
                        kn = min(QUAD, hbl - k0)
                        fe = kn * P
                        b0 = boff + k0  # block within window
                        kv4 = kvt_[:, k0 : k0 + kn, :]
                        q4 = qg[:, b0 : b0 + kn, :]
                        ea4 = eat[:, b0 * P : b0 * P + fe]

                        ehp = eh_ps.tile([P, QUAD * P], F32, space="PSUM", tag="ehp")
                        for k in range(kn):
                            nc.tensor.matmul(
                                out=ehp[:, k * P : (k + 1) * P],
                                lhsT=ea4[:, k * P : (k + 1) * P],
                                rhs=w4b[:, 3 * DIM : 4 * DIM],
                                start=True, stop=True,
                            )
                        ehs = stream.tile([P, fe], BF16, tag="ehs")
                        nc.scalar.activation(
                            out=ehs[:], in_=ehp[:, 0:fe], func=ACTF.Copy
                        )

                        kq = stream.tile([P, fe], BF16, tag="kq")
                        nc.vector.tensor_tensor(
                            out=kq[:].rearrange("p (q n) -> p q n", q=kn),
                            in0=kv4[:, :, 0:DIM],
                            in1=q4,
                            op=ALU.mult,
                        )
                        t3 = stream.tile([P, fe], BF16, tag="t3")
                        nc.vector.tensor_tensor(
                            out=t3[:], in0=kq[:], in1=ehs[:], op=ALU.mult
                        )
                        sraw = stream.tile([P, kn, H], F32, tag="sraw")
                        nc.vector.tensor_reduce(
                            out=sraw[:],
                            in_=t3[:].rearrange("p (q h d) -> p q h d", q=kn, h=H),
                            axis=mybir.AxisListType.X,
                            op=ALU.add,
                        )
                        sclip = stream.tile([P, kn, H], F32, tag="sclip")
                        nc.vector.tensor_scalar(
                            out=sclip[:], in0=sraw[:],
                            scalar1=20.0, scalar2=-20.0,
                            op0=ALU.min, op1=ALU.max,
                        )
                        rhs4 = stream.tile([P, kn, DIM + H], BF16, tag="rhs4")
                        nc.scalar.activation(
                            out=rhs4[:, :, DIM : DIM + H],
                            in_=sclip[:], func=ACTF.Exp, scale=0.25,
                        )
                        nc.vector.tensor_tensor(
                            out=rhs4[:, :, 0:DIM].rearrange(
                                "p q (h d) -> p q h d", h=H
                            ),
                            in0=kv4[:, :, DIM : 2 * DIM].rearrange(
                                "p q (h d) -> p q h d", h=H
                            ),
                            in1=rhs4[:, :, DIM : DIM + H]
                            .unsqueeze(-1)
                            .to_broadcast((P, kn, H, D)),
                            op=ALU.mult,
                        )
                        oh = stream.tile([P, kn, P], BF16, tag="oh")
                        nc.vector.tensor_tensor(
                            out=oh[:],
                            in0=drel_all[:, w * bpg + b0 : w * bpg + b0 + kn]
                            .unsqueeze(-1)
                            .to_broadcast((P, kn, P)),
                            in1=iot[:, 0:kn, :],
                            op=ALU.is_equal,
                        )
                        for k in range(kn):
                            nc.tensor.matmul(
                                out=acc[:],
                                lhsT=oh[:, k, :],
                                rhs=rhs4[:, k, :],
                                start=(nsteps == 0),
                                stop=(nsteps == total_steps - 1),
                            )
                            nsteps += 1

                # ---- window epilogue ----
                zeps = evac.tile([P, H], F32, tag="zeps")
                nc.vector.tensor_scalar_add(zeps[:], acc[:, DIM : DIM + H], 1e-6)
                rec = evac.tile([P, H], F32, tag="rec")
                nc.vector.reciprocal(out=rec[:], in_=zeps[:])
                hout = evac.tile([P, DIM], F32, tag="hout")
                nc.vector.tensor_tensor(
                    out=hout[:].rearrange("p (h d) -> p h d", h=H),
                    in0=acc[:, 0:DIM].rearrange("p (h d) -> p h d", h=H),
                    in1=rec[:].unsqueeze(-1).to_broadcast((P, H, D)),
                    op=ALU.mult,
                )
                nc.sync.dma_start(out=out[w * P : (w + 1) * P, :], in_=hout[:])

        for _rep in range(repeat):
            emit_rep()

    nc.compile()

    # Each DMASW sem lane is locked to one SWDGE queue at runtime; the Tile
    # scheduler assigns lanes in scheduled (not emission) order, so fix the
    # queue AFTER compile: lane k -> queue k % 4.
    from concourse.tile_sem_assignment import PROC_NAME_TO_IDX

    base = PROC_NAME_TO_IDX["DMASW0"]
    for b in nc.m.functions[0].blocks:
        for i in b.instructions:
            if isinstance(i, mybir.InstDMAGatherAnt):
                lane = getattr(i, "bass_scheduled_proc", None)
                if lane is not None:
                    i.queue_num = (lane - base) % 4
    return nc


def _wrap16(x):
    """int16 idx stream -> [128, n/16] gather-index tile (8 Q7 cores)."""
    n = x.shape[0]
    t = x.reshape(n // 16, 16).T.astype(np.int16)
    return np.ascontiguousarray(np.tile(t, (8, 1)))


def shard_inputs(h, edge_attr, WQ, WK, WV, WE, edge_index, n_cores, n_nodes):
    src = np.asarray(edge_index[0]).astype(np.int64)
    dst = np.asarray(edge_index[1]).astype(np.int64)
    e = src.shape[0]
    nwin = 392
    wpc = nwin // n_cores
    n_pad = nwin * P

    win = dst >> 7
    half = (src >= HALF).astype(np.int64)
    nlo = np.bincount(win[half == 0], minlength=nwin)
    nhi = np.bincount(win[half == 1], minlength=nwin)
    bl = max(1, int(math.ceil(nlo.max() / P)))
    bh = max(1, int(math.ceil(nhi.max() / P)))
    bpg = bl + bh
    spw = bpg * P

    group = win * 2 + half
    order = np.argsort(group, kind="stable")
    g_sorted = group[order]
    counts = np.bincount(group, minlength=2 * nwin)
    starts = np.concatenate(([0], np.cumsum(counts)[:-1]))
    within = np.arange(e, dtype=np.int64) - starts[g_sorted]
    w_s = g_sorted >> 1
    h_s = g_sorted & 1
    slot = w_s * spw + np.where(h_s == 1, bl * P + within, within)

    nslots = nwin * spw
    slot_eid = np.full(nslots, -1, dtype=np.int64)
    slot_eid[slot] = order

    mask = slot_eid >= 0
    eid_safe = np.where(mask, slot_eid, 0)
    src_slot = np.where(mask, src[eid_safe], 0)
    dst_slot = np.where(mask, dst[eid_safe], 0)
    win_of_slot = np.arange(nslots, dtype=np.int64) // spw
    half_of_slot = (np.arange(nslots, dtype=np.int64) % spw) >= bl * P

    # gather indices (pad slots -> row 0 of the relevant half-table)
    src16 = np.where(half_of_slot, src_slot - HALF, src_slot)
    src16 = np.where(mask, src16, 0).astype(np.int16)
    core_of_win = win_of_slot // wpc
    dst_loc = dst_slot - core_of_win * (wpc * P)
    dst_loc = np.where(mask, dst_loc, (win_of_slot % wpc) * P).astype(np.int16)
    drel = np.where(mask, dst_slot - win_of_slot * P, -1).astype(np.float32)

    ea = np.asarray(edge_attr, dtype=np.float32).astype(ml_dtypes.bfloat16)
    hT = np.zeros((DIM, n_pad), dtype=ml_dtypes.bfloat16)
    hT[:, :n_nodes] = np.asarray(h, np.float32).T.astype(ml_dtypes.bfloat16)
    w4 = np.ascontiguousarray(
        np.concatenate(
            [np.asarray(WQ), np.asarray(WK), np.asarray(WV), np.asarray(WE)],
            axis=1,
        ).astype(np.float32)
    )
    iota4 = np.tile(np.arange(P, dtype=np.float32), (P, QUAD)).astype(
        ml_dtypes.bfloat16
    )

    spc = wpc * spw  # slots per core
    in_maps = []
    for c in range(n_cores):
        sl = slice(c * spc, (c + 1) * spc)
        eid_c = slot_eid[sl]
        m_c = eid_c >= 0
        ea_c = np.zeros((spc, DIM), dtype=ml_dtypes.bfloat16)
        ea_c[m_c] = ea[eid_c[m_c]]
        s3 = src16[sl].reshape(wpc, spw)
        in_maps.append(
            {
                "eaT": np.ascontiguousarray(ea_c.T),
                "sloT": _wrap16(np.ascontiguousarray(s3[:, : bl * P]).reshape(-1)),
                "shiT": _wrap16(np.ascontiguousarray(s3[:, bl * P :]).reshape(-1)),
                "qixT": _wrap16(dst_loc[sl]),
                "drelT": np.ascontiguousarray(
                    drel[sl].reshape(wpc * bpg, P).T
                ).astype(ml_dtypes.bfloat16),
                "hTb": hT,
                "hqTb": np.ascontiguousarray(hT[:, c * wpc * P : (c + 1) * wpc * P]),
                "w4": w4,
                "iota4": np.ascontiguousarray(iota4),
            }
        )
    return in_maps, wpc, bl, bh


def kernel(h, edge_attr, WQ, WK, WV, WE, edge_index):
    global LAST_EXEC_NS, LAST_NC, LAST_IN_MAPS
    n_nodes = np.asarray(h).shape[0]
    in_maps, wpc, bl, bh = shard_inputs(
        h, edge_attr, WQ, WK, WV, WE, edge_index, N_CORES, n_nodes
    )
    nc = build_program(N_CORES, wpc, bl, bh)
    LAST_NC, LAST_IN_MAPS = nc, in_maps
    res = run_bass_kernel_spmd(nc, in_maps, list(range(N_CORES)))
    LAST_EXEC_NS = res.exec_time_ns
    outs = [np.asarray(res.results[c]["out"]) for c in range(N_CORES)]
    full = np.concatenate(outs, axis=0)[:n_nodes]
    return full.astype(np.float32)
